# revision 33
# baseline (speedup 1.0000x reference)
"""Trainium2 Bass kernel for causal multi-head attention (nn_Attention).

Reference computation (fp32):
    q = einsum('bsd,hde->bshe', x, W_Q) + b_Q
    k = einsum('bsd,hde->bshe', x, W_K) + b_K
    v = einsum('bsd,hde->bshe', x, W_V) + b_V
    scores  = einsum('bqhe,bkhe->bhqk', q, k) / sqrt(64)   (causal masked)
    pattern = softmax(scores, axis=-1)
    z       = einsum('bhqk,bkhe->bqhe', pattern, v)
    out     = einsum('bqhe,hed->bqd', z, W_O) + b_O

Shapes: x[2, 2048, 1024], 16 heads x 64 d_head.

Sharding: 8 cores = 2 batches x 4 head-groups (4 heads per core).
Each core computes its batch's partial output sum_h Z_h @ W_O[h] as
out^T [1024, 2048]; host sums the 4 head-group partials per batch.

Device-side layout (everything transposed so no on-chip transposes):
    Q^T/K^T: [he, s] head-major on partitions (chunk = 2 heads of 64)
    S^T    : [k, q]  computed as K^T_chunk.T @ Q^T  (2-head row-tiled)
    Z'^T   : [65, q] = [V | ones].T @ pattern^T  (row 64 = softmax denom)
    out^T  : [d, q]  = W_O^T @ Z^T

Causality: upper-triangle tiles skipped; diagonal tiles exp'd only on
the valid q-range and masked with one [128,128] triangular 0/1 mask.
Diagonal tiles of the two head streams share one exp instruction
(diag_pair), and full tiles are exp'd two-at-a-time (paired_exp): ACT
per-instruction overhead (~293ns of the ~(N+352)/1.2ns cost) is a real
HW co-bottleneck with PE issue.  Output goes through a staged SBUF
tensor (out_stage) and is written bf16 with one DMA per 128-row block
-- 4KB contiguous DRAM runs per partition instead of 4x 1KB, cutting
DGE descriptor-issue cost (~20us/iter measured for the scattered
version).

Biases: softmax is invariant to per-q additive shifts, so Q/K biases
reduce to a per-(head,k) bias  bQ.(K_k + bK)/sqrt(d)  applied inside the
exp (zeros when b_Q == 0).  V bias folds into an effective output bias
b_O + sum_h b_V[h] @ W_O[h]  added on the host (pattern rows sum to 1).
"""

import os
from contextlib import ExitStack

import numpy as np
import ml_dtypes

import concourse.bass as bass
import concourse.mybir as mybir
import concourse.tile as tile
from concourse import bacc
from concourse.bass_utils import run_bass_kernel_spmd

# problem constants (hardcoded per harness contract)
B, S, D, NH, DH = 2, 2048, 1024, 16, 64
N_CORES = 8
GPB = N_CORES // B           # head-groups per batch = 4
HPC = NH // GPB              # 4 heads per core
HE = HPC * DH                # 256
P = 128
QT = 512                     # q tile (free dim of score tiles)
NQ = S // QT                 # 4
KC = S // P                  # 16 k chunks
DC = D // P                  # 8 d chunks
HC = HE // P                 # 2 he chunks (2 heads each)

BF16 = mybir.dt.bfloat16
F32 = mybir.dt.float32

LAST_RESULTS = None  # BassKernelResults of the most recent run (for test.py)
_NC_CACHE = None


def _build_bass(reps=1, pp_bufs=2, ps_bufs=2, pz_bufs=2, pat_bufs=10,
                order="v8", lookahead=8, paired_exp=True, att1_desc=False,
                tail_act_copy=False, gpsimd_mask=False, early_release=False,
                xt_ndma=3, warmup_mms=0, ablate="", out_bf16=False,
                out_ndma=3, diag_pair=False, out_stage=False,
                mm_bcast=False):
    nc = bacc.Bacc()
    LOOKAHEAD = lookahead
    PAIRED_EXP = paired_exp
    ABLATE = set(ablate.split(",")) if ablate else set()
    DIAG_PAIR = diag_pair
    assert not (diag_pair and not paired_exp), \
        "diag_pair shares one exp across streams; needs zero score bias"
    OUT_DT = BF16 if (out_bf16 or out_stage) else F32
    ATT1_QNS = [3, 2, 1, 0] if att1_desc else [0, 1, 2, 3]
    TAIL_ACT_COPY = tail_act_copy
    GPSIMD_MASK = gpsimd_mask
    EARLY_RELEASE = early_release
    WARMUP_MMS = warmup_mms
    XT_SPLITS = {8: [(i, i + 1) for i in range(8)],
                 3: [(0, 3), (3, 6), (6, 8)],
                 6: [(0, 2), (2, 4), (4, 6), (6, 7), (7, 8)],
                 31: [(0, 3), (3, 5), (5, 8)],   # lighter scalar queue
                 32: [(0, 4), (4, 6), (6, 8)],   # heavier sync queue
                 }[xt_ndma]
    xT = nc.dram_tensor("xT", [D, S], BF16, kind="ExternalInput")
    wq = nc.dram_tensor("wq", [D, HE], BF16, kind="ExternalInput")
    wk = nc.dram_tensor("wk", [D, HE], BF16, kind="ExternalInput")
    wv = nc.dram_tensor("wv", [D, HE], BF16, kind="ExternalInput")
    wo = nc.dram_tensor("wo", [HE, D], BF16, kind="ExternalInput")
    # host pre-transposed to [P, HPC*KC] so the DMA is contiguous per
    # partition (element-strided layouts explode into per-element DMA
    # descriptors: ~100us of SWDGE descriptor generation)
    sbias = nc.dram_tensor("sbias", [P, HPC * KC], F32, kind="ExternalInput")
    outT = nc.dram_tensor("outT", [D, S], OUT_DT, kind="ExternalOutput")

    inv_sqrt = 1.0 / float(np.sqrt(DH))

    with tile.TileContext(nc) as tc, ExitStack() as ctx:
        const = ctx.enter_context(tc.tile_pool(name="const", bufs=1))
        sb = ctx.enter_context(tc.tile_pool(name="sb", bufs=6))
        pat = ctx.enter_context(tc.tile_pool(name="pat", bufs=pat_bufs))
        nrm = ctx.enter_context(tc.tile_pool(name="nrm", bufs=6))
        pp = ctx.enter_context(tc.tile_pool(name="pp", bufs=pp_bufs, space="PSUM"))
        ps = ctx.enter_context(tc.tile_pool(name="ps", bufs=ps_bufs, space="PSUM"))
        pz = ctx.enter_context(tc.tile_pool(name="pz", bufs=pz_bufs, space="PSUM"))

        # ---- resident SBUF tensors ----
        xT_sb = const.tile([P, DC, S], BF16, tag="xT_sb")      # x^T [d, s]
        wq_sb = const.tile([P, DC, HE], BF16, tag="wq_sb")
        wk_sb = const.tile([P, DC, HE], BF16, tag="wk_sb")
        wv_sb = const.tile([P, DC, HE], BF16, tag="wv_sb")
        wo_sb = const.tile([P, HC, D], BF16, tag="wo_sb")      # W_O [he, d]
        sb_sb = const.tile([P, HPC, KC], F32, tag="sb_sb")     # score bias
        qt_sb = const.tile([P, HC, S], BF16, tag="qt_sb")      # Q^T [he, q]
        kt_sb = const.tile([P, HC, S], BF16, tag="kt_sb")      # K^T [he, k]
        vp_sb = const.tile([P, KC, HPC, DH + 1], BF16, tag="vp_sb")
        zt_sb = const.tile([P, HC, S], BF16, tag="zt_sb")      # Z^T [he, q]
        tri = const.tile([P, P], BF16, tag="tri")              # keep mask
        # staged output: all out^T tiles collect here so each DRAM row is
        # written by ONE dma (4KB contiguous run per partition instead of
        # 4x 1KB runs -> 4x fewer DGE descriptors)
        ot_all = (const.tile([P, DC, S], BF16, tag="ot_all", name="ot_all")
                  if out_stage else None)
        # mm_bcast: [1,64] ones column; rcb = ones.T @ rc broadcasts the
        # reciprocal row across 64 partitions on PE (~0.2us) instead of
        # the gpsimd partition_broadcast library call (~1.8us latency on
        # the normalize->out_proj critical chain)
        ones64 = (const.tile([1, 64], BF16, tag="ones64", name="ones64")
                  if mm_bcast else None)
        if mm_bcast:
            nc.vector.memset(ones64[:], 1.0)

        # ---- input DMAs: weights first (small), xT spread over several
        # DGE queues (different issuing engines) to parallelize the 4MB ----
        nc.scalar.dma_start(wq_sb[:], wq.rearrange("(c p) e -> p c e", p=P))
        nc.scalar.dma_start(wk_sb[:], wk.rearrange("(c p) e -> p c e", p=P))
        nc.gpsimd.dma_start(wv_sb[:], wv.rearrange("(c p) e -> p c e", p=P))
        nc.gpsimd.dma_start(wo_sb[:], wo.rearrange("(c p) d -> p c d", p=P))
        nc.sync.dma_start(sb_sb[:], sbias.rearrange("p (h c) -> p h c", h=HPC))
        dma_engs = [nc.sync, nc.scalar, nc.gpsimd]
        out_dma_engs = [nc.sync, nc.scalar, nc.gpsimd, nc.vector,
                        nc.tensor][:out_ndma]
        for i, (d0, d1) in enumerate(XT_SPLITS):
            dma_engs[i % len(dma_engs)].dma_start(
                xT_sb[:, d0:d1],
                xT[d0 * P:d1 * P, :].rearrange("(c p) s -> p c s", p=P))

        # ---- constants: tri[p, j] = 1 if j >= p else 0; V' ones column ----
        nc.gpsimd.memset(tri[:], 1.0)
        nc.gpsimd.affine_select(
            out=tri[:], in_=tri[:], compare_op=mybir.AluOpType.is_ge,
            fill=0.0, base=0, pattern=[[1, P]], channel_multiplier=-1,
        )
        nc.vector.memset(vp_sb[:, :, :, DH:DH + 1], 1.0)

        def qk_proj(hp, qns=range(NQ)):
            """Q^T/K^T for head pair hp: out[he_chunk, q] = W^T x."""
            for qn in qns:
                for w_sb, dst in ((wq_sb, qt_sb), (wk_sb, kt_sb)):
                    acc = pp.tile([P, QT], F32, tag="pp", name="acc_qk")
                    for dc in range(DC):
                        nc.tensor.matmul(
                            acc[:],
                            w_sb[:, dc, hp * P:(hp + 1) * P],
                            xT_sb[:, dc, qn * QT:(qn + 1) * QT],
                            start=(dc == 0), stop=(dc == DC - 1),
                        )
                    nc.vector.tensor_copy(
                        dst[:, hp, qn * QT:(qn + 1) * QT], acc[:])

        def v_proj(kcs=range(KC)):
            """V' [k_chunk, h, e|1] = x^T_chunk^T W_V."""
            for kc in kcs:
                acc = pp.tile([P, HE], F32, tag="pp", name="acc_v")
                for dc in range(DC):
                    nc.tensor.matmul(
                        acc[:],
                        xT_sb[:, dc, kc * P:(kc + 1) * P],
                        wv_sb[:, dc, :],
                        start=(dc == 0), stop=(dc == DC - 1),
                    )
                nc.vector.tensor_copy(
                    vp_sb[:, kc, :, 0:DH],
                    acc[:].rearrange("p (h e) -> p h e", h=HPC),
                )

        def attention(hp, tail=None, qns=range(NQ), hps=None, pace=None,
                      prelude=None, mid=None):
            """Scores -> exp -> PV (2-head row-tiled score matmuls;
            causal tiles only, diagonal tiles sliced).  `hps` merges
            several head pairs into one qn loop (more independent PE
            work to hide ACT-bound stretches)."""
            if hps is None:
                hps = [hp]
            for qn in qns:
                zac = {(p_, i): pz.tile([DH + 1, QT], F32, tag="pz",
                                        name=f"zac{p_}_{i}")
                       for p_ in hps for i in range(2)}
                last_kc = 4 * qn + 3
                # units: 1 exp instruction each; full (below-diagonal)
                # tiles are paired two-per-exp across two PSUM banks when
                # the score bias is unused (PAIRED_EXP).
                units = []
                for p_ in hps:
                    for h2 in range(2):
                        kcs = list(range(last_kc + 1))
                        full = [k for k in kcs if k < 4 * qn]
                        diag = [k for k in kcs if k >= 4 * qn]
                        if PAIRED_EXP:
                            grp = [full[i:i + 2]
                                   for i in range(0, len(full), 2)]
                        else:
                            grp = [[k] for k in full]
                        units += [(tuple(g), p_, h2) for g in grp]
                        if not DIAG_PAIR:
                            units += [((k,), p_, h2) for k in diag]
                    if DIAG_PAIR:
                        # one unit per diagonal k-chunk covering BOTH h2
                        # streams of this head pair: 2 score matmuls into
                        # one [P,2,QT] bank pair, a single shared exp
                        diag = [k for k in range(last_kc + 1)
                                if k >= 4 * qn]
                        units += [((k,), p_, 2) for k in diag]
                # interleave across head streams for pipelining variety
                units.sort(key=lambda u: (u[0][0], u[1], min(u[2], 1)))

                pts = {}

                def score_exp(u):
                    kcg, p_, h2 = u
                    if h2 == 2:  # DIAG_PAIR: both streams, one diag chunk
                        kc = kcg[0]
                        lo = (kc - 4 * qn) * P
                        sps = ps.tile([P, 2, QT], F32, tag="ps", name="sps")
                        pt = pat.tile([P, 2, QT], BF16, tag="pat", name="pt")
                        for j in range(2):
                            rb = j * 64
                            nc.tensor.matmul(
                                sps[:, j, lo:QT],
                                kt_sb[rb:rb + 64, p_, kc * P:(kc + 1) * P],
                                qt_sb[rb:rb + 64, p_,
                                      qn * QT + lo:(qn + 1) * QT],
                                start=True, stop=True,
                                tile_position=(rb, 0),
                            )
                        nc.scalar.activation(
                            pt[:, :, lo:QT], sps[:, :, lo:QT],
                            mybir.ActivationFunctionType.Exp,
                            scale=inv_sqrt,
                        )
                        for j in range(2):
                            if GPSIMD_MASK:
                                nc.gpsimd.affine_select(
                                    out=pt[:, j, lo:lo + P],
                                    in_=pt[:, j, lo:lo + P],
                                    compare_op=mybir.AluOpType.is_ge,
                                    fill=0.0, base=0, pattern=[[1, P]],
                                    channel_multiplier=-1,
                                )
                            else:
                                nc.vector.tensor_mul(
                                    pt[:, j, lo:lo + P],
                                    pt[:, j, lo:lo + P], tri[:],
                                )
                        pts[u] = pt
                        return
                    rowb = h2 * 64
                    h = 2 * p_ + h2
                    sps = ps.tile([P, 2, QT], F32, tag="ps", name="sps")
                    pt = pat.tile([P, 2, QT], BF16, tag="pat", name="pt")
                    los = []
                    for j, kc in enumerate(kcg):
                        o = kc - 4 * qn
                        lo = max(o, 0) * P
                        los.append(lo)
                        nc.tensor.matmul(
                            sps[:, j, lo:QT],
                            kt_sb[rowb:rowb + 64, p_, kc * P:(kc + 1) * P],
                            qt_sb[rowb:rowb + 64, p_,
                                  qn * QT + lo:(qn + 1) * QT],
                            start=True, stop=True,
                            tile_position=(rowb, 0),
                        )
                    if len(kcg) == 2:
                        nc.scalar.activation(
                            pt[:, :, :], sps[:, :, :],
                            mybir.ActivationFunctionType.Exp,
                            scale=inv_sqrt,
                        )
                    else:
                        kc, lo = kcg[0], los[0]
                        o = kc - 4 * qn
                        nc.scalar.activation(
                            pt[:, 0, lo:QT], sps[:, 0, lo:QT],
                            mybir.ActivationFunctionType.Exp,
                            bias=sb_sb[:, h, kc:kc + 1], scale=inv_sqrt,
                        )
                        if o >= 0:
                            if GPSIMD_MASK:
                                nc.gpsimd.affine_select(
                                    out=pt[:, 0, lo:lo + P],
                                    in_=pt[:, 0, lo:lo + P],
                                    compare_op=mybir.AluOpType.is_ge,
                                    fill=0.0, base=0, pattern=[[1, P]],
                                    channel_multiplier=-1,
                                )
                            else:
                                nc.vector.tensor_mul(
                                    pt[:, 0, lo:lo + P], pt[:, 0, lo:lo + P],
                                    tri[:],
                                )
                    pts[u] = pt

                def pv(u):
                    kcg, p_, h2 = u
                    pt = pts.pop(u)
                    if h2 == 2:  # DIAG_PAIR: PV for both streams
                        kc = kcg[0]
                        lo = (kc - 4 * qn) * P
                        for j in range(2):
                            nc.tensor.matmul(
                                zac[p_, j][:, lo:QT],
                                vp_sb[:, kc, 2 * p_ + j, :],
                                pt[:, j, lo:QT],
                                start=(kc == 0), stop=(kc == last_kc),
                            )
                        return
                    for j, kc in enumerate(kcg):
                        lo = max(kc - 4 * qn, 0) * P
                        nc.tensor.matmul(
                            zac[p_, h2][:, lo:QT],
                            vp_sb[:, kc, 2 * p_ + h2, :],
                            pt[:, j, lo:QT],
                            start=(kc == 0), stop=(kc == last_kc),
                        )

                # software-pipelined emission: scores/exp run `la` units
                # ahead of PV so in-order PE never stalls on an exp dep
                # with ready score work behind it.
                la = min(LOOKAHEAD, len(units))
                for u in units[:la]:
                    score_exp(u)
                if prelude is not None:  # e.g. V proj after the score
                    prelude()            # prefix so exps start earliest
                    prelude = None
                mid_at = max(1, len(units) // 3)
                for i, u in enumerate(units):
                    if i + la < len(units):
                        score_exp(units[i + la])
                    pv(u)
                    if i == mid_at and mid is not None:
                        mid(qn)  # pre-flush next q-chunk's projections
                    if pace is not None:
                        pace()
                # normalize: Z = Z_unnorm * (1/denom), denom = row DH of zac
                for p_ in hps:
                    for h2 in range(2):
                        if EARLY_RELEASE:
                            # copy the accumulator to SBUF first so the
                            # PSUM bank frees for the next q-chunk's PV
                            # without waiting for the normalize chain
                            zcp = nrm.tile([DH + 1, QT], F32, tag="zcp",
                                           name="zcp")
                            nc.vector.tensor_copy(zcp[:], zac[p_, h2][:])
                            src = zcp
                        else:
                            src = zac[p_, h2]
                        if mm_bcast:
                            # broadcast 1/denom across 64 partitions via a
                            # rank-1 PE matmul instead of the ~1.8us gpsimd
                            # library call (bf16 reciprocal: ~0.4% on the
                            # normalization, inside the error budget)
                            rc = nrm.tile([1, QT], BF16, tag="rc",
                                          name="rc")
                            with nc.allow_low_precision(
                                    reason="1/denom bf16: ~0.4% on the "
                                    "softmax norm, inside 2e-2 budget"):
                                nc.vector.reciprocal(
                                    rc[:], src[DH:DH + 1, :])
                            rcb = pp.tile([64, QT], F32, tag="pp",
                                          name="rcb_ps")
                            nc.tensor.matmul(
                                rcb[:], ones64[:], rc[:],
                                start=True, stop=True,
                            )
                        else:
                            rc = nrm.tile([1, QT], F32, tag="rc", name="rc")
                            nc.vector.reciprocal(rc[:], src[DH:DH + 1, :])
                            rcb = nrm.tile([64, QT], F32, tag="rcb",
                                           name="rcb")
                            nc.gpsimd.partition_broadcast(rcb[:], rc[:])
                        nc.vector.tensor_mul(
                            zt_sb[h2 * 64:h2 * 64 + 64, p_,
                                  qn * QT:(qn + 1) * QT],
                            src[0:DH, :], rcb[:],
                        )
                if tail is not None:
                    tail(qn)

        done_qns = set()

        def out_proj(qn):
            """out^T[d, q-chunk] = W_O^T Z^T (needs both hp chunks)."""
            done_qns.add(qn)
            for m in range(DC):
                acc = pp.tile([P, QT], F32, tag="pp", name="acc_o")
                for hc in range(HC):
                    nc.tensor.matmul(
                        acc[:],
                        wo_sb[:, hc, m * P:(m + 1) * P],
                        zt_sb[:, hc, qn * QT:(qn + 1) * QT],
                        start=(hc == 0), stop=(hc == HC - 1),
                    )
                if out_stage:
                    nc.vector.tensor_copy(
                        ot_all[:, m, qn * QT:(qn + 1) * QT], acc[:])
                    if len(done_qns) == NQ and "noout" not in ABLATE:
                        out_dma_engs[m % len(out_dma_engs)].dma_start(
                            outT[m * P:(m + 1) * P, :], ot_all[:, m, :])
                    continue
                ot = sb.tile([P, QT], OUT_DT, tag="ot", name="ot")
                if TAIL_ACT_COPY and qn == NQ - 1:
                    # last q-chunk: ACT is idle by now (exps done); take
                    # its copies off the DVE tail
                    nc.scalar.copy(ot[:], acc[:])
                else:
                    nc.vector.tensor_copy(ot[:], acc[:])
                if "noout" not in ABLATE:
                    out_dma_engs[m % len(out_dma_engs)].dma_start(
                        outT[m * P:(m + 1) * P, qn * QT:(qn + 1) * QT],
                        ot[:],
                    )

        def run_v8():
            done_qns.clear()  # fresh per rep (staged-out DMA gating)
            """Single software-pipelined stream: projection matmul groups
            ("fillers") are woven between attention tiles so the in-order
            PE stream always has ready work during ACT-bound stretches.
            Fillers carry required-before tags; they are force-flushed
            before the attention q-chunk that needs them."""
            # PE warmup: the HAM clock gate runs the array at half rate
            # until ~3us of sustained activity; burn dummy matmuls on the
            # already-arrived wq while the xT DMA is still in flight so
            # the real projections start at full clock.  The accumulator
            # is never read (released right after the group).
            if WARMUP_MMS:
                wu = pp.tile([P, HE], F32, tag="pp", name="warmup")
                for i in range(WARMUP_MMS):
                    nc.tensor.matmul(
                        wu[:], wq_sb[:, 0, 0:P], wq_sb[:, 0, :],
                        start=(i == 0), stop=(i == WARMUP_MMS - 1),
                    )

            # startup: what attn0(qn=0) needs (V emitted via prelude,
            # after qn0's score prefix, so the first exps start sooner)
            qk_proj(0, qns=[0])

            fillers = []  # (emit_fn, (hp, qn, late) required-before)
            for kc in range(4, KC):
                fillers.append(
                    (lambda kc=kc: v_proj(kcs=[kc]), (0, kc // 4, 1)))
            for qn in range(1, NQ):
                fillers.append(
                    (lambda qn=qn: qk_proj(0, qns=[qn]), (0, qn, 0)))
            for qn in range(NQ):
                fillers.append(
                    (lambda qn=qn: qk_proj(1, qns=[qn]), (1, qn, 0)))
            # flush_required scans in order: must be sorted by the tag
            fillers.sort(key=lambda f: f[1])

            state = {"i": 0}

            def flush_upto(tag):
                while state["i"] < len(fillers):
                    fn, req = fillers[state["i"]]
                    if req is not None and req <= tag:
                        fn()
                        state["i"] += 1
                    else:
                        break

            def flush_required(hp, qn):
                flush_upto((hp, qn, 9))

            # rough pacing: one filler group per `stride` attention tiles
            total_tiles = 2 * sum(2 * (4 * qn + 4) for qn in range(NQ))
            stride = max(1, total_tiles // (len(fillers) + 1))
            tick = {"n": 0}

            def pace():
                tick["n"] += 1
                if tick["n"] % stride == 0 and state["i"] < len(fillers):
                    fn, _req = fillers[state["i"]]
                    fn()
                    state["i"] += 1

            def attn_pass(hp, tail=None, qns=range(NQ), prelude=None):
                def mid(qn):
                    # emit the NEXT q-chunk's projections mid-stream so
                    # its scores can issue right at the boundary (the
                    # next chunk here is (hp, qn+1) or (hp+1, 0))
                    nxt = (hp, qn + 1, 0) if qn + 1 < NQ else (hp + 1, 0, 0)
                    flush_upto(nxt)

                for qn in qns:
                    flush_required(hp, qn)
                    attention(hp, qns=[qn], tail=tail, pace=pace,
                              prelude=prelude, mid=mid)
                    prelude = None

            attn_pass(0, prelude=lambda: v_proj(kcs=range(0, 4)))
            # descending: the 4x-shorter q-chunk (qn=0) lands last, so
            # the post-last-exp tail (PV+normalize+out_proj) is minimal
            attn_pass(1, tail=out_proj, qns=ATT1_QNS)
            while state["i"] < len(fillers):
                fillers[state["i"]][0]()
                state["i"] += 1

        # emission order = scheduler priority.  Pipelined start: per-qn
        # V/QK projections feed hp0's attention immediately so ACT (the
        # co-bottleneck) starts exp'ing ~11us in instead of ~42us; hp1's
        # projections after hp0's attention fill PE idle during its
        # ACT-bound stretch; out_proj per q-chunk interleaves into hp1's
        # attention as Z completes.
        for _ in range(reps):  # reps>1 only for timing harnesses
            if order == "v8":
                run_v8()
            elif order == "v6":
                # both head pairs merged per q-chunk; out_proj per qn
                for qn in range(NQ):
                    qk_proj(0, qns=[qn])
                    qk_proj(1, qns=[qn])
                    v_proj(kcs=range(4 * qn, 4 * qn + 4))
                    attention(0, hps=[0, 1], qns=[qn], tail=out_proj)
            elif order == "v5":
                # hp1 projections interleaved into hp0's attention (fills
                # PE while ACT-bound); attention(1) descending so the
                # shortest q-chunk (fewest k tiles) is the tail.
                for qn in range(NQ):
                    qk_proj(0, qns=[qn])
                    v_proj(kcs=range(4 * qn, 4 * qn + 4))
                    attention(0, qns=[qn])
                    qk_proj(1, qns=[NQ - 1 - qn])
                attention(1, tail=out_proj, qns=[3, 2, 1, 0])
            elif order == "v4":
                for qn in range(NQ):
                    qk_proj(0, qns=[qn])
                    v_proj(kcs=range(4 * qn, 4 * qn + 4))
                    attention(0, qns=[qn])
                qk_proj(1)
                attention(1, tail=out_proj)
            elif order == "v3":
                for qn in range(NQ):
                    v_proj(kcs=range(4 * qn, 4 * qn + 4))
                    qk_proj(0, qns=[qn])
                    attention(0, qns=[qn])
                qk_proj(1)
                attention(1, tail=out_proj)
            elif order.startswith("phase"):  # bisection probes
                n = int(order[5:])
                v_proj()
                if n >= 2:
                    qk_proj(0)
                    qk_proj(1)
                if n >= 3:
                    attention(0)
                if n >= 4:
                    attention(1, tail=None)
                if n >= 5:
                    for qn in range(NQ):
                        out_proj(qn)
            else:
                v_proj()
                qk_proj(0)
                attention(0)
                qk_proj(1)
                attention(1, tail=out_proj)

    # Bacc compile: wait-splitting (TRN2 allows 1 wait/instr), library
    # loads for partition_broadcast, InstISA codegen, reg alloc.
    nc.finalize()
    return nc


def _build_kwargs(b_Q):
    """Build flags shared by kernel() and the test harness's timing
    artifact.  exp pairing (and the diag cross-stream pairing) share one
    activation instruction across k-chunks/streams — only valid when the
    score bias is zero (b_Q == 0)."""
    paired = not bool(np.any(b_Q))
    # gpsimd_mask: diagonal causal masks run on the near-idle Pool engine
    # (affine_select) instead of DVE — on HW the engines quasi-serialize
    # on cross-engine deps, so relieving DVE pays directly (-9us/iter)
    return dict(paired_exp=paired, diag_pair=paired, out_stage=True,
                gpsimd_mask=True)


def make_in_maps(x, W_Q, b_Q, W_K, b_K, W_V, W_O):
    """Host-side sharding: per-core input dict (bf16 device layouts)."""
    bf = ml_dtypes.bfloat16
    inv_sqrt = 1.0 / np.sqrt(DH)

    # per-(head, k) score bias (exact for softmax; zero when b_Q == 0)
    if np.any(b_Q):
        k_full = np.einsum("bsd,hde->bhse", x, W_K) + b_K[None, :, None, :]
        sb_full = np.einsum("he,bhse->bhs", b_Q, k_full) * inv_sqrt
        sb_full = sb_full.astype(np.float32)
    else:
        sb_full = np.zeros((B, NH, S), dtype=np.float32)

    in_maps = []
    for c in range(N_CORES):
        b = c // GPB
        h0 = (c % GPB) * HPC
        hs = slice(h0, h0 + HPC)
        in_maps.append({
            "xT": np.ascontiguousarray(x[b].T).astype(bf),
            "wq": np.ascontiguousarray(
                W_Q[hs].transpose(1, 0, 2).reshape(D, HE)).astype(bf),
            "wk": np.ascontiguousarray(
                W_K[hs].transpose(1, 0, 2).reshape(D, HE)).astype(bf),
            "wv": np.ascontiguousarray(
                W_V[hs].transpose(1, 0, 2).reshape(D, HE)).astype(bf),
            "wo": np.ascontiguousarray(W_O[hs].reshape(HE, D)).astype(bf),
            # [HPC, S] -> [P, HPC*KC] with k = c*128 + p
            "sbias": np.ascontiguousarray(
                sb_full[b, hs].reshape(HPC, KC, P).transpose(2, 0, 1)
                .reshape(P, HPC * KC)),
        })
    return in_maps


def unshard(results, W_O, b_V, b_O):
    """Host-side gather: sum head-group partials per batch + bias fold."""
    b_O_eff = b_O + np.einsum(
        "e,ed->d", b_V.reshape(-1).astype(np.float32),
        W_O.reshape(NH * DH, D).astype(np.float32))
    out = np.zeros((B, S, D), dtype=np.float32)
    for c in range(N_CORES):
        out[c // GPB] += results[c]["outT"].T.astype(np.float32)
    out += b_O_eff.astype(np.float32)
    return out


def kernel(normalized_resid_pre, W_Q, b_Q, W_K, b_K, W_V, b_V, W_O, b_O):
    global LAST_RESULTS, _NC_CACHE
    x = np.asarray(normalized_resid_pre, dtype=np.float32)
    W_Q, W_K, W_V, W_O = (np.asarray(w, dtype=np.float32)
                          for w in (W_Q, W_K, W_V, W_O))
    b_Q, b_K, b_V, b_O = (np.asarray(v, dtype=np.float32)
                          for v in (b_Q, b_K, b_V, b_O))

    in_maps = make_in_maps(x, W_Q, b_Q, W_K, b_K, W_V, W_O)

    if _NC_CACHE is None:
        _NC_CACHE = _build_bass(**_build_kwargs(b_Q))

    trace = bool(int(os.environ.get("KERNEL_TRACE", "0")))
    res = run_bass_kernel_spmd(
        _NC_CACHE, in_maps, core_ids=list(range(N_CORES)), trace=trace,
    )
    LAST_RESULTS = res
    return unshard(res.results, W_O, b_V, b_O)

